# revision 28
# baseline (speedup 1.0000x reference)
"""Trainium2 Bass kernel for nn_DTKSA (sparse top-k channel attention).

Self-contained: kernel(**inputs) takes the FULL float32 inputs (as produced by
the oracle's setup_inputs) and returns the FULL float32 output, running an SPMD
Bass/Tile kernel on 8 NeuronCores.

Sharding: spatial row-bands everywhere. Each core owns 24 full-res rows
(12 pooled rows) plus a 2-full-res-row halo for the depthwise conv. The
channel attention (48x48 per head) contracts over the FULL spatial extent, so
each core computes a per-head Gram partial G_h = [q_h;k_h] @ [q_h;k_h]^T over
its spatial slice and ONE AllReduce (~300KB) sums them; diag(G) provides the
L2 norms. Everything else (softmax coefficient matrix, CW @ v, gelu, proj,
2x nearest upsample) is spatially local and computed redundantly-or-locally.
"""

import threading

import numpy as np
import ml_dtypes

import bass_rust
import concourse.bass as bass
import concourse.mybir as mybir
import concourse.tile as tile
from concourse.vector_clock import ScopedClock
from concourse.bass_utils import run_bass_kernel_spmd

# ----------------------------------------------------------------------------
# Tile tail-drain workaround: this walrus build rejects >1 sync-wait on the
# SP/CTRL Drain that TileContext emits at exit. Keep one wait on the drain and
# give each extra wait its own follow-up Drain (1-wait CTRL drains are what
# all_engine_barrier itself emits, so they are known-good).
# ----------------------------------------------------------------------------


def _patched_drain_and_barrier(self, tick_clock, wait_clock):
    nc = self.nc
    drain_inst = nc.sync.drain()
    wait_clock.add_sem_waits(
        drain_inst.ins, ScopedClock({None: tick_clock.global_clock})
    )
    si = drain_inst.ins.sync_info
    conds = list(si.on_wait or []) if si is not None else []
    if len(conds) > 1:
        si.on_wait = conds[:1]
        for cond in conds[1:]:
            extra = nc.sync.drain()
            extra.ins.sync_info = bass_rust.SyncInfo(on_wait=[cond], on_update=[])
    nc.all_engine_barrier()
    assert self.sems is not None
    popped = nc._tile_sem_poison_stack.pop()
    assert popped is self._sem_poison
    nc.clear_and_free_semaphores(list(self.sems.allocated().values()))
    nc.all_engine_barrier()


tile.TileContext._drain_and_barrier = _patched_drain_and_barrier

# This walrus build also caps the number of sync-wait commands a single
# instruction may carry (Tile can attach more). Split any excess waits onto
# same-engine NOPs inserted immediately before the instruction.
_WAIT_CAP = 1


def _split_sync_waits(nc, cap=_WAIT_CAP):
    for fn in nc.m.functions:
        for blk in fn.blocks:
            insts = list(blk.instructions)
            out, n_added = [], 0
            for ins in insts:
                si = getattr(ins, "sync_info", None)
                waits = list(si.on_wait or []) if si is not None else []
                if len(waits) > cap:
                    si.on_wait = waits[:cap]
                    rest = waits[cap:]
                    for i in range(0, len(rest), cap):
                        nop = mybir.InstNoOp(
                            name=f"{ins.name}_w{i}",
                            engine=ins.engine,
                            ins=[], outs=[],
                            sync_info=bass_rust.SyncInfo(
                                on_wait=rest[i:i + cap], on_update=[]),
                        )
                        out.append(nop)
                        n_added += 1
                out.append(ins)
            if n_added:
                blk.instructions = out

# ----------------------------------------------------------------------------
# Problem constants (hardcoded per the harness contract).
# ----------------------------------------------------------------------------
NC = 8                  # cores
DIM = 384               # channels
HEADS = 8
C = DIM // HEADS        # 48 channels/head
HF, WF = 192, 192       # full-res spatial
HP, WP = 96, 96         # pooled spatial
RPC = HF // NC          # 24 full-res rows per core
PRC = HP // NC          # 12 pooled rows per core
XR = RPC + 4            # 28 full-res rows incl. 2-row halo each side
NSP = XR * WF           # 5376 spatial columns in stage-1
NT = 448                # stage-1 matmul N-tile (5376 = 12*448)
NQT = 3                 # N-tiles per x quarter (1344 = 3*448)
PR = PRC + 2            # 14 pooled rows incl. 1 halo row each side
PW = WP + 2             # 98: pooled row stride with zero pad cols
MB = DIM * 3 // 128     # 9 channel blocks of 128 in qkv
TAPS = [(dy, dx) for dy in (-1, 0, 1) for dx in (-1, 0, 1)]
KKS = [C // 2, C * 2 // 3, C * 3 // 4, C * 4 // 5]   # 24, 32, 36, 38
NEG = -1.0e30

F32 = mybir.dt.float32
F32R = mybir.dt.float32r
F16 = mybir.dt.float16
BF16 = mybir.dt.bfloat16
AX = mybir.AxisListType
ALU = mybir.AluOpType
ACTF = mybir.ActivationFunctionType


def build_kernel():
    nc = bass.Bass(target_bir_lowering=False, debug=False)

    x_in = nc.declare_dram_parameter("x_slice", [3, 8, 128, 672], F16, isOutput=False)
    wqkv_in = nc.declare_dram_parameter("wqkvT", [3, 128, 1152], F16, isOutput=False)
    dw_in = nc.declare_dram_parameter("dwdiag", [128, 81, 128], F16, isOutput=False)
    wproj_in = nc.declare_dram_parameter("wprojT", [4, 96, 384], BF16, isOutput=False)
    bqkv_in = nc.declare_dram_parameter("bqkv", [128, 9, 3], F32, isOutput=False)
    bdw_in = nc.declare_dram_parameter("bdw", [128, 9], F32, isOutput=False)
    bproj_in = nc.declare_dram_parameter("bproj", [128, 3], F32, isOutput=False)
    ident_in = nc.declare_dram_parameter("ident", [128, 128], F16, isOutput=False)
    temp_in = nc.declare_dram_parameter("tempb", [48, 8], F32, isOutput=False)
    atile_in = nc.declare_dram_parameter("atile", [96, 16], F32, isOutput=False)
    y_out = nc.declare_dram_parameter("y_slice", [3, 128, RPC * WF], BF16, isOutput=True)

    with tile.TileContext(nc) as tc:
        with (
            tc.tile_pool(name="persist", bufs=1) as persist,
            tc.tile_pool(name="dram", bufs=1, space="DRAM") as dram,
        ):
            # ---- persistent SBUF tiles -----------------------------------
            pooled = [persist.tile([128, PR * PW], F16, tag=f"pooled{m}", name=f"pooled{m}")
                      for m in range(MB)]
            ident = persist.tile([128, 128], F16, tag="ident")
            dwdiag = persist.tile([128, 81, 128], F16, tag="dwdiag")
            bqkv = persist.tile([128, 9, 3], F32, tag="bqkv")
            bdw = persist.tile([128, 9], F32, tag="bdw")
            bproj = persist.tile([128, 3], F32, tag="bproj")
            tempb = persist.tile([48, 8], F32, tag="tempb")
            atile = persist.tile([96, 16], F32, tag="atile")

            nc.sync.dma_start(ident[:], ident_in[:, :])
            nc.sync.dma_start(bqkv[:], bqkv_in[:, :, :])
            nc.sync.dma_start(bdw[:], bdw_in[:, :])
            nc.sync.dma_start(bproj[:], bproj_in[:, :])
            nc.sync.dma_start(tempb[:], temp_in[:, :])
            nc.sync.dma_start(atile[:], atile_in[:, :])

            # ================= Phase A: qkv conv + 2x2 max pool ===========
            with (
                tc.tile_pool(name="ph_a", bufs=1) as ph_a,
                tc.tile_pool(name="ps_a", bufs=7, space="PSUM") as ps_a,
            ):
                x_sb = [ph_a.tile([128, NSP], F16, tag=f"x{k}",
                                  name=f"x{k}") for k in range(3)]
                wq = ph_a.tile([128, 3, 1152], F16, tag="wq")
                wq_src = wqkv_in.rearrange("k p f -> p k f")
                for mq in range(MB):
                    for k in range(3):
                        nc.sync.dma_start(
                            wq[:, k, mq * 128:(mq + 1) * 128],
                            wq_src[:, k, mq * 128:(mq + 1) * 128])
                for q in range(8):
                    for k in range(3):
                        nc.sync.dma_start(
                            x_sb[k][:, q * 672:(q + 1) * 672],
                            x_in[k, q, :, :])
                for m in range(MB):
                    nc.sync.dma_start(dwdiag[:, m * 9:(m + 1) * 9, :],
                                      dw_in[:, m * 9:(m + 1) * 9, :])

                for m in range(MB):
                  with nc.named_scope("phA"):
                    pl3 = pooled[m].rearrange("p (r c) -> p r c", c=PW)
                    nc.vector.memset(pl3[:, :, 0:1], 0.0)
                    nc.vector.memset(pl3[:, :, 97:98], 0.0)
                    # 14 row-pair N-tiles of 384; the 2x2 max pool collapses
                    # each tile to one pooled row with a single XY reduce.
                    # k outer / tile inner so the stationary weights reload
                    # only 3x per half instead of per tile (walrus runs with
                    # ldw-opt off, so every matmul carries its own LDWEIGHTS).
                    for half in range(2):
                        nt0 = half * 7
                        pss = [ps_a.tile([128, 384], F32, tag="ps_a",
                                         name="ps_a") for _ in range(7)]
                        for k in range(3):
                            for i in range(7):
                                c0 = (nt0 + i) * 384
                                nc.tensor.matmul(
                                    pss[i][:],
                                    lhsT=wq[:, k, m * 128:(m + 1) * 128],
                                    rhs=x_sb[k][:, c0:c0 + 384],
                                    start=(k == 0),
                                    stop=(k == 2),
                                )
                        for i in range(7):
                            ps4 = pss[i].rearrange("p (r x c) -> p x r c",
                                                   r=2, c=2)
                            nc.vector.tensor_reduce(
                                pl3[:, nt0 + i, 1:97], ps4,
                                axis=AX.XY, op=ALU.max)
                    # qkv bias (post-pool; halo rows use per-core masked bias)
                    nc.scalar.activation(
                        pl3[:, 1:13, 1:97], pl3[:, 1:13, 1:97],
                        ACTF.Identity, bias=bqkv[:, m, 0:1], scale=1.0)
                    nc.scalar.activation(
                        pl3[:, 0:1, 1:97], pl3[:, 0:1, 1:97],
                        ACTF.Identity, bias=bqkv[:, m, 1:2], scale=1.0)
                    nc.scalar.activation(
                        pl3[:, 13:14, 1:97], pl3[:, 13:14, 1:97],
                        ACTF.Identity, bias=bqkv[:, m, 2:3], scale=1.0)

            # ---- late-lifetime persistent tiles (phases B..F) ------------
            with tc.tile_pool(name="late", bufs=1) as late:
                qk_buf = [late.tile([128, PRC * WP], F16, tag=f"qk{m}",
                                    name=f"qk{m}") for m in range(6)]
                qkT = [late.tile([128, 768], F16, tag=f"qkT{j}",
                                 name=f"qkT{j}") for j in range(9)]
                v_pair = [late.tile([96, PRC * WP], BF16, tag=f"vp{p}",
                                    name=f"vp{p}") for p in range(4)]
                y_pair = [late.tile([96, PRC * WP], BF16, tag=f"yp{p}",
                                    name=f"yp{p}") for p in range(4)]
                g_buf = late.tile([48, 400], F32, tag="gbuf")
                wproj = late.tile([96, 4, 384], BF16, tag="wproj")
                identf = late.tile([96, 96], F32, tag="identf")
                nc.sync.dma_start(
                    wproj[:], wproj_in.rearrange("k p f -> p k f"))
                nc.vector.tensor_copy(identf[:], ident[0:96, 0:96])

                # ===== Phase B/C: depthwise 3x3, Gram + AllReduce =============
                # Order: q,k blocks -> transposes -> Gram -> AllReduce, THEN
                # the v blocks, so the collective latency hides behind the
                # v-block depthwise matmuls.
                with (
                    tc.tile_pool(name="ph_b", bufs=2) as ph_b,
                    tc.tile_pool(name="ps_b", bufs=2, space="PSUM") as ps_b,
                ):
                    identf128 = ph_b.tile([128, 128], F16, tag="if128")
                    nc.scalar.copy(identf128[:], ident[:])

                    def dw_block(m, ps_tp):
                        pl3 = pooled[m].rearrange("p (r c) -> p r c", c=PW)
                        pss = [ps_b.tile([128, 4 * WP], F32, tag=f"ps_b{s}",
                                         name=f"ps_b{s}") for s in range(3)]
                        for t, (dy, dx) in enumerate(TAPS):
                            for s in range(3):
                                r0 = 1 + 4 * s + dy
                                nc.tensor.matmul(
                                    pss[s],
                                    lhsT=dwdiag[:, m * 9 + t, :],
                                    rhs=pl3[:, r0:r0 + 4, 1 + dx:97 + dx],
                                    start=(t == 0),
                                    stop=(t == 8),
                                )
                        if m < 6:
                            # q,k: evict + b_dw into qk_buf, then PE-transpose
                            for s in range(3):
                                nc.scalar.activation(
                                    qk_buf[m][:, s * 384:(s + 1) * 384],
                                    pss[s][:], ACTF.Identity,
                                    bias=bdw[:, m:m+1], scale=1.0)
                            for jj in range(9):
                                tp = ps_tp.tile([128, 128], F16, tag="tp",
                                                name="tp")
                                nc.tensor.transpose(
                                    tp[:],
                                    qk_buf[m][:, jj * 128:(jj + 1) * 128],
                                    identf128[:])
                                nc.vector.tensor_copy(
                                    qkT[jj][:, m * 128:(m + 1) * 128], tp[:])
                        else:
                            # v: evict + b_dw, DMA-rearrange into head pairs
                            vs = ph_b.tile([128, PRC * WP], BF16, tag="vstage")
                            for s in range(3):
                                nc.scalar.activation(
                                    vs[:, s * 384:(s + 1) * 384], pss[s][:],
                                    ACTF.Identity, bias=bdw[:, m:m+1],
                                    scale=1.0)
                            base = (m - 6) * 128
                            lo_pair, lo_off = divmod(base, 96)
                            take0 = 96 - lo_off if lo_off else 96
                            nc.sync.dma_start(
                                v_pair[lo_pair][lo_off:lo_off + take0, :],
                                vs[0:take0, :])
                            if take0 < 128:
                                nc.sync.dma_start(
                                    v_pair[lo_pair + 1][0:128 - take0, :],
                                    vs[take0:128, :])

                    with tc.tile_pool(name="ps_t", bufs=2,
                                      space="PSUM") as ps_t:
                        for m in range(6):
                            with nc.named_scope("dw_qk"):
                                dw_block(m, ps_t)

                    # local sum-of-squares of q,k channels (free-dim accum
                    # on ACT) -> staged into g_buf cols [384:400] through DRAM
                    # (partition regroup); summed globally by the AllReduce.
                    nc.enter_named_scope("gram", False)
                    sq_acc = late.tile([128, 6], F32, tag="sq_acc")
                    sq_scr2 = late.tile([128, PRC * WP], F32,
                                        tag="sq_scr2")
                    for m6 in range(6):
                        nc.scalar.activation(
                            sq_scr2[:],
                            qk_buf[m6][:],
                            ACTF.Square,
                            accum_out=sq_acc[:, m6:m6 + 1])
                    # regroup via DRAM: store channel-contiguous, then
                    # reload as (48 ch-in-head, 16 head-cols):
                    # flat[c] with c = 128*b + p; dst col h = q head h,
                    # col 8+h = k head h; src offset = i + 48*col.
                    sq_d = dram.tile([768], F32, name="sq_d")
                    nc.sync.dma_start(
                        sq_d.rearrange("(b p) -> p b", p=128), sq_acc[:])
                    nc.sync.dma_start(
                        g_buf[:, 384:400],
                        sq_d.rearrange("(col i) -> i col", i=48))

                    with tc.tile_pool(name="ps_g", bufs=2,
                                      space="PSUM") as ps_g:
                        # per head: A = q_h^T @ k_h over all 9 chunks
                        for h in range(HEADS):
                            gp1 = ps_g.tile([48, 48], F32, tag="gp1",
                                            name="gp1")
                            for jj in range(9):
                                nc.tensor.matmul(
                                    gp1,
                                    lhsT=qkT[jj][:, h * 48:(h + 1) * 48],
                                    rhs=qkT[jj][:, 384 + h * 48:
                                                384 + (h + 1) * 48],
                                    start=(jj == 0), stop=(jj == 8))
                            nc.scalar.copy(
                                g_buf[:, h * 48:(h + 1) * 48], gp1[:])

                        nc.leave_named_scope("gram", 0, False)
                        nc.enter_named_scope("allreduce", False)
                        cc_in = dram.tile([48, 400], F32)
                        cc_out = dram.tile([48, 400], F32,
                                           addr_space="Shared")
                        nc.sync.dma_start(cc_in[:], g_buf[:])
                        nc.gpsimd.collective_compute(
                            "AllReduce", ALU.add,
                            replica_groups=[list(range(NC))],
                            ins=[cc_in.opt()], outs=[cc_out.opt()],
                        )
                        nc.sync.dma_start(g_buf[:], cc_out[:])

                    nc.leave_named_scope("allreduce", 0, False)
                    for m in range(6, 9):
                        with nc.named_scope("dw_v"):
                            dw_block(m, None)

                # ================= Phase D: attention coefficient matrices ====
                with (
                    tc.tile_pool(name="ph_d", bufs=1) as ph_d,
                    tc.tile_pool(name="ps_d", bufs=2, space="PSUM") as ps_d,
                    nc.named_scope("phD"),
                ):
                    # norms: sumsq = g_buf[:, 384:400] (48, 16):
                    # col h = ||q_i||^2 head h, col 8+h = ||k_i||^2 head h
                    sumsq = ph_d.tile([48, 16], F32, tag="sumsq")
                    nc.vector.tensor_scalar_max(
                        sumsq[:], g_buf[:, 384:400], 1.0e-24)
                    nrm = ph_d.tile([48, 16], F32, tag="nrm")
                    nc.scalar.sqrt(nrm[:], sumsq[:])
                    rns = ph_d.tile([48, 16], F32, tag="rns")
                    nc.vector.reciprocal(rns[:], nrm[:])

                    # fold temperature into the k-norm columns up front
                    nc.vector.tensor_tensor(
                        rns[:, 8:16], rns[:, 8:16], tempb[:, 0:8],
                        op=ALU.mult)
                    # transpose rns -> (16, 48); stage each k-norm row at
                    # partition 0 so it is a legal matmul rhs, then broadcast
                    # along partitions via K=1 matmuls with a ones column.
                    rtp = ps_d.tile([48, 48], F32, tag="rtp")
                    nc.tensor.transpose(rtp[0:16, :], rns[:],
                                        identf[0:48, 0:48])
                    rnsT = ph_d.tile([16, 48], F32, tag="rnsT")
                    nc.scalar.copy(rnsT[:], rtp[0:16, 0:48])
                    krow = ph_d.tile([1, 384], F32, tag="krow")
                    for h in range(HEADS):
                        nc.sync.dma_start(krow[0:1, h * 48:(h + 1) * 48],
                                          rnsT[8 + h:9 + h, :])

                    rkb = ph_d.tile([96, 192], F32, tag="rkb")
                    ones1 = ph_d.tile([1, 48], F32, tag="ones1")
                    nc.vector.memset(ones1[:], 1.0)
                    rk_stage = ph_d.tile([48, 192], F32, tag="rk_stage")
                    for half in range(2):
                        rkps = ps_d.tile([48, 192], F32, tag="rkps",
                                         name=f"rkps{half}")
                        for g in range(4):
                            h = 2 * g + half
                            nc.tensor.matmul(
                                rkps[:, g * 48:(g + 1) * 48],
                                lhsT=ones1[:],
                                rhs=krow[0:1, h * 48:(h + 1) * 48],
                                start=True, stop=True)
                        dst = rkb if half == 0 else rk_stage
                        nc.scalar.copy(dst[0:48, :], rkps[:])
                    nc.sync.dma_start(rkb[48:96, :], rk_stage[:])
                    # q-norms, partition-aligned: rqb (96, 4)
                    rqb = ph_d.tile([96, 4], F32, tag="rqb")
                    rns2 = rns.rearrange("p (g x) -> p g x", x=2)
                    nc.sync.dma_start(rqb[0:48, :], rns2[:, 0:4, 0])
                    nc.sync.dma_start(rqb[48:96, :], rns2[:, 0:4, 1])

                    # A packed (96, 4*48): group g = heads (2g | 2g+1)
                    a_all = ph_d.tile([96, 192], F32, tag="a_all")
                    for g in range(4):
                        for half in range(2):
                            h = 2 * g + half
                            nc.sync.dma_start(
                                a_all[half * 48:half * 48 + 48,
                                      g * 48:(g + 1) * 48],
                                g_buf[0:48, h * 48:(h + 1) * 48])
                    a3 = a_all.rearrange("p (g c) -> p g c", c=48)
                    nc.vector.tensor_tensor(
                        a3, a3,
                        rqb[:, :, None].to_broadcast([96, 4, 48]),
                        op=ALU.mult)
                    nc.vector.tensor_tensor(a_all[:], a_all[:], rkb[:], op=ALU.mult)

                    # top-40 per row via 5 rounds of max8 + match_replace
                    srt = ph_d.tile([96, 4, 40], F32, tag="sorted")
                    scr = ph_d.tile([96, 192], F32, tag="scratch")
                    for g in range(4):
                        src = a_all[:, g * 48:(g + 1) * 48]
                        dst = scr[:, g * 48:(g + 1) * 48]
                        for r in range(5):
                            nc.vector.max(srt[:, g, r * 8:(r + 1) * 8],
                                          src if r == 0 else dst)
                            nc.vector.match_replace(
                                out=dst,
                                in_to_replace=srt[:, g, r * 8:(r + 1) * 8],
                                in_values=src if r == 0 else dst,
                                imm_value=NEG)

                    # E over sorted values, partial sums -> coefs a_b / s_b
                    es = ph_d.tile([96, 4, 40], F32, tag="esort")
                    rmax_b = srt[:, :, 0:1].to_broadcast([96, 4, 40])
                    nc.vector.tensor_tensor(es[:], srt[:], rmax_b, op=ALU.subtract)
                    nc.scalar.activation(es[:], es[:], ACTF.Exp)
                    sall = ph_d.tile([96, 4, 4], F32, tag="sall")
                    nc.vector.tensor_reduce(
                        sall[:, :, 0], es[:, :, 0:KKS[0]], axis=AX.X, op=ALU.add)
                    for b in range(1, 4):
                        nc.vector.tensor_reduce(
                            sall[:, :, b], es[:, :, KKS[b - 1]:KKS[b]],
                            axis=AX.X, op=ALU.add)
                        nc.vector.tensor_add(
                            sall[:, :, b], sall[:, :, b], sall[:, :, b - 1])
                    call = ph_d.tile([96, 4, 4], F32, tag="call")
                    nc.vector.reciprocal(call[:], sall[:])
                    nc.vector.tensor_tensor(
                        call[:], call[:],
                        atile.rearrange("p (g b) -> p g b", b=4),
                        op=ALU.mult)

                    # msum = sum_b c_b * [A >= t_b]; CW = exp(A - rowmax) * msum
                    msum = ph_d.tile([96, 192], F32, tag="msum")
                    mb_t = ph_d.tile([96, 192], F32, tag="mb")
                    for b in range(4):
                        tgt = msum if b == 0 else mb_t
                        tgt3 = tgt.rearrange("p (g c) -> p g c", c=48)
                        nc.vector.tensor_tensor(
                            tgt3,
                            a_all.rearrange("p (g c) -> p g c", c=48),
                            srt[:, :, KKS[b] - 1:KKS[b]].to_broadcast([96, 4, 48]),
                            op=ALU.is_ge)
                        nc.vector.tensor_tensor(
                            tgt3, tgt3,
                            call[:, :, b:b + 1].to_broadcast([96, 4, 48]),
                            op=ALU.mult)
                        if b > 0:
                            nc.vector.tensor_add(msum[:], msum[:], mb_t[:])

                    cw = ph_d.tile([96, 192], F32, tag="cw")
                    nc.vector.tensor_tensor(
                        cw.rearrange("p (g c) -> p g c", c=48),
                        a_all.rearrange("p (g c) -> p g c", c=48),
                        srt[:, :, 0:1].to_broadcast([96, 4, 48]),
                        op=ALU.subtract)
                    nc.scalar.activation(cw[:], cw[:], ACTF.Exp)
                    nc.vector.tensor_tensor(cw[:], cw[:], msum[:], op=ALU.mult)

                    # per pair: block-diag CW -> transpose -> lhsT; out = CW @ v
                    with tc.tile_pool(name="ph_e", bufs=2) as ph_e, \
                            nc.named_scope("phE"):
                        for g in range(4):
                            bd = ph_e.tile([96, 96], F32, tag="bdiag")
                            nc.vector.memset(bd[:], 0.0)
                            nc.vector.tensor_copy(
                                bd[0:48, 0:48], cw[0:48, g * 48:(g + 1) * 48])
                            nc.sync.dma_start(
                                bd[48:96, 48:96], cw[48:96, g * 48:(g + 1) * 48])
                            tps = ps_d.tile([96, 96], F32, tag="tps")
                            nc.tensor.transpose(tps[:], bd[:], identf[:])
                            cwt = ph_e.tile([96, 96], BF16, tag="cwt")
                            nc.vector.tensor_copy(cwt[:], tps[:])
                            for s in range(3):
                                ops = ps_d.tile([96, 384], F32, tag="ops")
                                nc.tensor.matmul(
                                    ops, lhsT=cwt[:],
                                    rhs=v_pair[g][:, s * 384:(s + 1) * 384],
                                    start=True, stop=True)
                                nc.scalar.activation(
                                    y_pair[g][:, s * 384:(s + 1) * 384], ops[:],
                                    ACTF.Gelu)

                # ================= Phase F: proj + upsample + output ==========
                with (
                    tc.tile_pool(name="ph_f", bufs=2) as ph_f,
                    tc.tile_pool(name="ps_f", bufs=4, space="PSUM") as ps_f,
                    nc.named_scope("phF"),
                ):
                    for m in range(3):
                        pj = ph_f.tile([128, PRC * WP], F32, tag="pj")
                        for s in range(3):
                            ps = ps_f.tile([128, 384], F32, tag="ps_f")
                            for k in range(4):
                                nc.tensor.matmul(
                                    ps,
                                    lhsT=wproj[:, k, m * 128:(m + 1) * 128],
                                    rhs=y_pair[k][:, s * 384:(s + 1) * 384],
                                    start=(k == 0),
                                    stop=(k == 3),
                                )
                            nc.scalar.activation(
                                pj[:, s * 384:(s + 1) * 384], ps[:],
                                ACTF.Identity, bias=bproj[:, m:m+1], scale=1.0)
                        # nearest-upsample x2 in x via step-0 read, x2 in y via
                        # writing each expanded row to two output rows
                        pjx = ph_f.tile([128, PRC * 2 * WP], BF16, tag="pjx")
                        pjx4 = pjx.rearrange("p (r c t) -> p r c t", c=WP, t=2)
                        pj3 = pj.rearrange("p (r c) -> p r c", c=WP)
                        nc.scalar.copy(
                            pjx4[:, 0:6],
                            pj3[:, 0:6, :, None].to_broadcast([128, 6, WP, 2]))
                        nc.vector.tensor_copy(
                            pjx4[:, 6:12],
                            pj3[:, 6:12, :, None].to_broadcast([128, 6, WP, 2]))
                        y3 = y_out.rearrange("m p (r two c) -> m p r two c",
                                             two=2, c=WF)
                        px3 = pjx.rearrange("p (r c) -> p r c", c=WF)
                        for rr in range(3):
                            r0, r1 = rr * 4, rr * 4 + 4
                            nc.sync.dma_start(y3[m, :, r0:r1, 0, :],
                                              px3[:, r0:r1, :])
                            nc.sync.dma_start(y3[m, :, r0:r1, 1, :],
                                              px3[:, r0:r1, :])

    _split_sync_waits(nc)
    return nc


# ----------------------------------------------------------------------------
# Host-side input preparation / sharding / gather
# ----------------------------------------------------------------------------

def _prep_core_inputs(x, w_qkv, b_qkv, w_dw, b_dw, w_proj, b_proj,
                      temperature, a1, a2, a3, a4):
    bf = ml_dtypes.bfloat16
    x = np.asarray(x, np.float32).reshape(DIM, HF, WF)
    w_qkv = np.asarray(w_qkv, np.float32)
    w_dw = np.asarray(w_dw, np.float32).reshape(3 * DIM, 3, 3)
    w_proj = np.asarray(w_proj, np.float32)

    wqkvT = np.ascontiguousarray(w_qkv.T).reshape(3, 128, 3 * DIM)
    wqkvT = wqkvT.astype(np.float16)
    wprojT = np.ascontiguousarray(w_proj.T).reshape(4, 96, DIM).astype(bf)

    # diagonal depthwise weight blocks: dwdiag[p, 9m+t, c] = w[m*128+p, t]*(p==c)
    dwdiag = np.zeros((128, 81, 128), np.float32)
    ar = np.arange(128)
    for m in range(MB):
        for t, (dy, dx) in enumerate(TAPS):
            dwdiag[ar, m * 9 + t, ar] = w_dw[m * 128 + ar, dy + 1, dx + 1]
    dwdiag = dwdiag.astype(np.float16)

    bq = np.asarray(b_qkv, np.float32).reshape(MB, 128)
    bd = np.asarray(b_dw, np.float32).reshape(MB, 128)
    bp = np.asarray(b_proj, np.float32).reshape(3, 128)

    ident = np.eye(128, dtype=np.float16)
    tempb = np.tile(np.asarray(temperature, np.float32).reshape(1, HEADS),
                    (48, 1))
    avec = np.array([np.float32(a1[0]), np.float32(a2[0]),
                     np.float32(a3[0]), np.float32(a4[0])], np.float32)
    atile = np.tile(avec, (96, 4)).astype(np.float32)

    # x: pad 2 halo rows of zeros top/bottom, slice per core
    xp = np.zeros((DIM, HF + 4, WF), np.float16)
    xp[:, 2:HF + 2, :] = x.astype(np.float16)
    in_maps = []
    for c in range(NC):
        xs = xp[:, c * RPC:c * RPC + XR, :]                  # (384, 28, 192)
        xs = xs.reshape(3, 128, XR * WF).reshape(3, 128, 8, 672)
        xs = np.ascontiguousarray(xs.transpose(0, 2, 1, 3))
        bqkv3 = np.stack([
            bq.T, bq.T * (1.0 if c > 0 else 0.0),
            bq.T * (1.0 if c < NC - 1 else 0.0)], axis=2)     # (128, 9, 3)
        in_maps.append({
            "x_slice": xs,
            "wqkvT": wqkvT,
            "dwdiag": dwdiag,
            "wprojT": wprojT,
            "bqkv": np.ascontiguousarray(bqkv3, np.float32),
            "bdw": np.ascontiguousarray(bd.T),
            "bproj": np.ascontiguousarray(bp.T),
            "ident": ident,
            "tempb": np.ascontiguousarray(tempb),
            "atile": atile,
        })
    return in_maps


_CACHE = {}
_CACHE_LOCK = threading.Lock()


def _make_runner():
    """Compile once; return a callable in_maps -> list[{name: array}].

    Mirrors concourse.bass2jax.run_bass_via_pjrt but caches the jitted
    executable so repeat kernel() calls do not recompile.
    """
    import jax
    import concourse.mybir as mybir
    from concourse import bass2jax
    from jax.experimental.shard_map import shard_map
    from jax.sharding import Mesh, PartitionSpec

    nc = build_kernel()
    bass2jax.install_neuronx_cc_hook()
    partition_name = (nc.partition_id_tensor.name
                      if nc.partition_id_tensor else None)
    in_names, out_names, out_avals, zero_outs = [], [], [], []
    for alloc in nc.m.functions[0].allocations:
        if not isinstance(alloc, mybir.MemoryLocationSet):
            continue
        name = alloc.memorylocations[0].name
        if alloc.kind == "ExternalInput":
            if name != partition_name:
                in_names.append(name)
        elif alloc.kind == "ExternalOutput":
            shape = tuple(alloc.tensor_shape)
            dtype = mybir.dt.np(alloc.dtype)
            out_names.append(name)
            out_avals.append(jax.core.ShapedArray(shape, dtype))
            zero_outs.append(np.zeros(shape, dtype))
    n_params = len(in_names)
    n_outs = len(out_avals)
    all_names = list(in_names) + list(out_names)
    if partition_name is not None:
        all_names.append(partition_name)
    donate = tuple(range(n_params, n_params + n_outs))

    def _body(*args):
        operands = list(args)
        if partition_name is not None:
            operands.append(bass2jax.partition_id_tensor())
        return tuple(bass2jax._bass_exec_p.bind(
            *operands,
            out_avals=tuple(out_avals),
            in_names=tuple(all_names),
            out_names=tuple(out_names),
            lowering_input_output_aliases=(),
            sim_require_finite=True,
            sim_require_nnan=True,
            nc=nc,
        ))

    devices = jax.devices()[:NC]
    mesh = Mesh(np.asarray(devices), ("core",))
    in_specs = (PartitionSpec("core"),) * (n_params + n_outs)
    out_specs = (PartitionSpec("core"),) * n_outs
    sharded = jax.jit(
        shard_map(_body, mesh=mesh, in_specs=in_specs, out_specs=out_specs,
                  check_rep=False),
        donate_argnums=donate, keep_unused=True)

    def _sync(t):
        return jax.lax.psum(t, "core")

    sync_fn = jax.jit(
        shard_map(_sync, mesh=mesh, in_specs=(PartitionSpec("core"),),
                  out_specs=PartitionSpec()))

    import jax.numpy as jnp
    sharding = jax.sharding.NamedSharding(mesh, PartitionSpec("core"))
    zeros_dev = jax.jit(
        lambda: tuple(
            jnp.zeros((NC * z.shape[0], *z.shape[1:]), z.dtype)
            for z in zero_outs),
        out_shardings=tuple(sharding for _ in zero_outs))

    def upload(in_maps):
        concat_in = [
            np.concatenate([np.asarray(in_maps[c][nm]) for c in range(NC)],
                           axis=0)
            for nm in in_names[:n_params]
        ]
        return [jax.device_put(a, sharding) for a in concat_in]

    import jax.numpy as _jnp
    sync_sharding = jax.sharding.NamedSharding(mesh, PartitionSpec("core"))
    sync_in = jax.device_put(np.zeros((NC, 8), np.float32), sync_sharding)

    def execute(dev_args):
        zouts = zeros_dev()
        sync_out = sync_fn(sync_in)
        out_arrs = sharded(*dev_args, *zouts)
        jax.block_until_ready(out_arrs)
        jax.block_until_ready(sync_out)
        return out_arrs

    def run(in_maps):
        out_arrs = execute(upload(in_maps))
        return [
            {nm: np.asarray(out_arrs[i]).reshape(NC, *out_avals[i].shape)[c]
             for i, nm in enumerate(out_names)}
            for c in range(NC)
        ]

    run.upload = upload
    run.execute = execute
    return run


def kernel(**inputs) -> np.ndarray:
    with _CACHE_LOCK:
        runner = _CACHE.get("runner")
        if runner is None:
            runner = _make_runner()
            _CACHE["runner"] = runner
    in_maps = _prep_core_inputs(**inputs)
    results = runner(in_maps)
    out = np.empty((1, DIM, HF, WF), np.float32)
    for c in range(NC):
        y = results[c]["y_slice"].astype(np.float32).reshape(DIM, RPC, WF)
        out[0, :, c * RPC:(c + 1) * RPC, :] = y
    return out

